# revision 1
# baseline (speedup 1.0000x reference)
"""GQA kernel for trn2, 8 NeuronCores, tensor-parallel over KV heads.

B=2, S=2048, H=2048, NQ=32, NKV=8, HD=64. Core c owns kv-head c and q-heads
4c..4c+3. Host pre-transposes x -> xT (B,H,S) and slices weights per core;
device computes q^T/kv^T projections, flash-style S^T -> exp -> PV with an
appended ones-column of V giving softmax denominators, scale by reciprocal,
output projection; host sums the 8 partial outputs + bo. Matmuls in float32r
(TF32-like, full rate, ~1e-4 rel err). Softmax max-subtraction is skipped:
scores ~ N(0,1), |max| ~ 6, exp is safe in fp32.
"""

import os
import sys

import numpy as np

sys.path.insert(0, "/opt/trn_rl_repo")

B, S, H = 2, 2048, 2048
NQ, NKV, HD = 32, 8, 64
G = NQ // NKV
QC = G * HD            # 256 q cols per core
P = 128
NCORES = 8

SQT = 512
N_SQT = S // SQT       # 4
N_SKC = S // P         # 16
N_HC = H // P          # 16
SH = 1024
N_OCT = H // SQT       # 4

_cached = {}


def _build_nc():
    from concourse import bacc
    import concourse.mybir as mybir
    import concourse.tile as tile
    from concourse.masks import make_identity

    f32 = mybir.dt.float32
    f32r = mybir.dt.float32r
    Exp = mybir.ActivationFunctionType.Exp
    mult = mybir.AluOpType.mult

    nc = bacc.Bacc("TRN2")
    xT_d = nc.declare_dram_parameter("xT", [B, H, S], f32, isOutput=False)
    wq_d = nc.declare_dram_parameter("wq", [H, QC], f32, isOutput=False)
    wkv_d = nc.declare_dram_parameter("wkv", [H, 2 * HD], f32, isOutput=False)
    wo_d = nc.declare_dram_parameter("wo", [QC, H], f32, isOutput=False)
    out_d = nc.declare_dram_parameter("out", [B, S, H], f32, isOutput=True)

    def rr(ap):
        return ap.bitcast(f32r)

    with tile.TileContext(nc) as tc:
        with (
            tc.tile_pool(name="weights", bufs=1) as wpool,
            tc.tile_pool(name="xstream", bufs=3) as xpool,
            tc.tile_pool(name="acts", bufs=1) as apool,
            tc.tile_pool(name="ptile", bufs=3) as ppool,
            tc.tile_pool(name="asmall", bufs=2) as aspool,
            tc.tile_pool(name="obuf", bufs=3) as opool,
            tc.tile_pool(name="psum", bufs=8, space="PSUM") as psum,
        ):
            wq_sb = wpool.tile([P, N_HC, QC], f32r)
            nc.sync.dma_start(wq_sb[:], rr(wq_d.rearrange("(hc p) c -> p hc c", p=P)))
            wkv_sb = wpool.tile([P, N_HC, 2 * HD], f32r)
            nc.sync.dma_start(wkv_sb[:], rr(wkv_d.rearrange("(hc p) c -> p hc c", p=P)))
            wo_sb = wpool.tile([P, 2, H], f32r)
            nc.sync.dma_start(wo_sb[:], rr(wo_d.rearrange("(c p) n -> p c n", p=P)))
            # eye(64) at partitions 64:128 (base partition must match v^T rows)
            ident = wpool.tile([P, HD], f32)
            nc.gpsimd.memset(ident[:], 0.0)
            make_identity(nc, ident[HD:P, :], nomemset=True)
            ones_t = wpool.tile([P, HD], f32r)
            nc.vector.memset(ones_t[:].bitcast(f32), 1.0)

            for b in range(B):
                # ---------- phase A: projections ----------
                qT = apool.tile([P, 2, S], f32r, tag="qT")
                qTo = apool.tile([HD, 2, S], f32r, tag="qTo")  # odd heads, base 0
                kvT = apool.tile([P, S], f32r, tag="kvT")      # k rows 0:64, v rows 64:128
                vp = apool.tile([P, N_SKC, HD + 1], f32r, tag="vp")

                for sh in range(2):
                    qp = [[psum.tile([P, SQT], f32, tag="ps", name=f"qp{cc}{st}")
                           for st in range(2)] for cc in range(2)]
                    kvp = [psum.tile([P, SQT], f32, tag="ps", name=f"kvp{st}")
                           for st in range(2)]
                    for hc in range(N_HC):
                        xt = xpool.tile([P, SH], f32r, tag="xt")
                        nc.sync.dma_start(
                            xt[:], rr(xT_d[b, hc * P:(hc + 1) * P, sh * SH:(sh + 1) * SH]))
                        for st in range(2):
                            rhs = xt[:, st * SQT:(st + 1) * SQT]
                            for cc in range(2):
                                nc.tensor.matmul(
                                    qp[cc][st], wq_sb[:, hc, cc * P:(cc + 1) * P],
                                    rhs, start=(hc == 0), stop=(hc == N_HC - 1))
                            nc.tensor.matmul(
                                kvp[st], wkv_sb[:, hc, :], rhs,
                                start=(hc == 0), stop=(hc == N_HC - 1))
                    for st in range(2):
                        s0 = sh * SH + st * SQT
                        for cc in range(2):
                            nc.vector.tensor_copy(qT[:, cc, s0:s0 + SQT], qp[cc][st])
                            nc.sync.dma_start(qTo[:, cc, s0:s0 + SQT],
                                              qT[HD:P, cc, s0:s0 + SQT])
                        nc.vector.tensor_copy(kvT[:, s0:s0 + SQT], kvp[st])

                # V' = [V | 1]: transpose v^T via PE, ones column for row-sums
                nc.vector.memset(vp[:, :, HD:HD + 1].bitcast(f32), 1.0)
                for t in range(N_SKC):
                    tp = psum.tile([P, SQT], f32, tag="ps")
                    nc.tensor.matmul(tp[:, :HD], kvT[HD:P, t * P:(t + 1) * P].bitcast(f32),
                                     ident[HD:P, :], is_transpose=True)
                    nc.vector.tensor_copy(vp[:, t, :HD], tp[:, :HD])

                # ---------- phase B: attention + out-proj ----------
                for sqt in range(N_SQT):
                    sq0 = sqt * SQT
                    aT = aspool.tile([P, 2, SQT], f32r, tag="aT")
                    for h in range(G):
                        cc, odd = h // 2, h % 2
                        outp = psum.tile([P, SQT], f32, tag="ps")
                        if odd:
                            qh = qTo[:, cc, sq0:sq0 + SQT]
                        else:
                            qh = qT[0:HD, cc, sq0:sq0 + SQT]
                        for sk in range(N_SKC):
                            sp = psum.tile([P, SQT], f32, tag="ps")
                            nc.tensor.matmul(
                                sp, kvT[0:HD, sk * P:(sk + 1) * P], qh,
                                start=True, stop=True)
                            pt = ppool.tile([P, SQT], f32r, tag="pt")
                            nc.scalar.activation(pt[:], sp, Exp, scale=0.125)
                            nc.tensor.matmul(
                                outp[0:HD + 1], vp[:, sk, :], pt[:],
                                start=(sk == 0), stop=(sk == N_SKC - 1))
                        # reciprocal of row-sum (row 64), broadcast via PE
                        rcp = aspool.tile([P, SQT], f32r, tag="rcp")
                        with nc.allow_low_precision(reason="f32r recip, 1e-4 ok"):
                            nc.vector.reciprocal(rcp[HD:HD + 1, :], outp[HD:HD + 1, :])
                        pbr = psum.tile([P, SQT], f32, tag="ps")
                        nc.tensor.matmul(pbr[0:HD, :], ones_t[HD:HD + 1, :],
                                         rcp[HD:HD + 1, :], start=True, stop=True)
                        rb = aspool.tile([HD, SQT], f32, tag="rb")
                        nc.vector.tensor_copy(rb[:], pbr[0:HD, :])
                        if odd:
                            tmp64 = aspool.tile([HD, SQT], f32r, tag="tmp64")
                            nc.vector.tensor_tensor(
                                tmp64[:], outp[0:HD, :], rb[:], op=mult)
                            nc.sync.dma_start(aT[HD:P, cc, :], tmp64[:])
                        else:
                            nc.vector.tensor_tensor(
                                aT[0:HD, cc, :], outp[0:HD, :], rb[:], op=mult)
                    for sqc in range(4):
                        row0 = sq0 + sqc * P
                        for oc in range(N_OCT):
                            op_ = psum.tile([P, SQT], f32, tag="ps")
                            for hdc in range(2):
                                nc.tensor.matmul(
                                    op_, aT[:, hdc, sqc * P:(sqc + 1) * P],
                                    wo_sb[:, hdc, oc * SQT:(oc + 1) * SQT],
                                    start=(hdc == 0), stop=(hdc == 1))
                            ob = opool.tile([P, SQT], f32, tag="ob")
                            nc.vector.tensor_copy(ob[:], op_)
                            nc.sync.dma_start(
                                out_d[b, row0:row0 + P, oc * SQT:(oc + 1) * SQT], ob[:])
    nc.compile()
    return nc


def kernel(**inputs):
    from concourse.bass_utils import run_bass_kernel_spmd

    x = np.asarray(inputs["x"], dtype=np.float32)
    Wq = np.asarray(inputs["Wq"], dtype=np.float32)
    Wk = np.asarray(inputs["Wk"], dtype=np.float32)
    Wv = np.asarray(inputs["Wv"], dtype=np.float32)
    Wo = np.asarray(inputs["Wo"], dtype=np.float32)
    bo = np.asarray(inputs["bo"], dtype=np.float32)

    xT = np.ascontiguousarray(x.transpose(0, 2, 1))
    in_maps = []
    for c in range(NCORES):
        wq_c = np.ascontiguousarray(Wq[:, c * QC:(c + 1) * QC])
        wkv_c = np.ascontiguousarray(
            np.concatenate([Wk[:, c * HD:(c + 1) * HD], Wv[:, c * HD:(c + 1) * HD]],
                           axis=1))
        wo_c = np.ascontiguousarray(Wo[c * QC:(c + 1) * QC, :])
        in_maps.append({"xT": xT, "wq": wq_c, "wkv": wkv_c, "wo": wo_c})

    if "nc" not in _cached:
        _cached["nc"] = _build_nc()
    trace = bool(int(os.environ.get("GQA_TRACE", "0")))
    res = run_bass_kernel_spmd(_cached["nc"], in_maps, list(range(NCORES)),
                               trace=trace)
    _cached["last_result"] = res
    out = res.results[0]["out"].astype(np.float32)
    for c in range(1, NCORES):
        out += res.results[c]["out"]
    out += bo
    return out



# revision 31
# speedup vs baseline: 2.6260x; 2.6260x over previous
"""GQA kernel for trn2, 8 NeuronCores, tensor-parallel over KV heads. v2.

B=2, S=2048, H=2048, NQ=32, NKV=8, HD=64. Core c owns kv-head c and q-heads
4c..4c+3. Host pre-transposes x -> xT (B,H,S) in bf16 and slices weights per
core (bf16); device computes q^T/kv^T projections (psum fp32), flash-style
S^T -> exp -> PV with an appended ones-column of V' giving softmax
denominators, reciprocal broadcast via one PE matmul per head-pair, output
projection; partial outputs written fp16, host sums the 8 partials + bo.

Layout tricks vs v1:
- bf16 operands everywhere on PE (same matmul rate as f32r, half the DMA/SBUF).
- Even head of a pair: V' = [V | ones] -> PV rows 0:64, denom row 64.
  Odd head: V' = [ones | 0 | V] -> denom row 0, PV rows 64:128. aT is then
  assembled with two partition-aligned DVE multiplies - no SBUF->SBUF DMA.
- k rows are duplicated to partitions 64:128 (one SBUF->SBUF DMA per batch)
  so odd-head score matmuls read q/k at base partition 64 directly.
- One broadcast matmul per head pair: bmask[64,0:64]=1 maps even rcp (row 64)
  to out rows 0:64; bmask[0,64:128]=1 maps odd rcp (row 0) to rows 64:128.
- exp processes 1024 columns per ACT instruction (2-bank psum score tiles).
- Softmax max-subtraction is skipped: scores ~ N(0,1), exp is safe in fp32.
"""

import os
import sys

import numpy as np

sys.path.insert(0, "/opt/trn_rl_repo")

B, S, H = 2, 2048, 2048
NQ, NKV, HD = 32, 8, 64
G = NQ // NKV
QC = G * HD            # 256 q cols per core
P = 128
NCORES = 8

SQT = 512
N_SQT = S // SQT       # 4
N_SKC = S // P         # 16
N_HC = H // P          # 16
SH = 1024

_cached = {}


def _build_nc():
    from concourse import bacc
    import concourse.mybir as mybir
    import concourse.tile as tile
    from concourse.masks import make_identity

    f32 = mybir.dt.float32
    f32r = mybir.dt.float32r
    bf16 = mybir.dt.bfloat16
    fp16 = mybir.dt.float16
    Exp = mybir.ActivationFunctionType.Exp
    mult = mybir.AluOpType.mult

    nc = bacc.Bacc("TRN2")
    xT_d = nc.declare_dram_parameter("xT", [B, H, S], bf16, isOutput=False)
    wq_d = nc.declare_dram_parameter("wq", [H, QC], bf16, isOutput=False)
    wkv_d = nc.declare_dram_parameter("wkv", [H, 2 * HD], bf16, isOutput=False)
    wo_d = nc.declare_dram_parameter("wo", [QC, H], bf16, isOutput=False)
    out_d = nc.declare_dram_parameter("out", [B, S, H], fp16, isOutput=True)

    with tile.TileContext(nc) as tc:
        with (
            tc.tile_pool(name="weights", bufs=1) as wpool,
            tc.tile_pool(name="xstream", bufs=2) as xpool,
            tc.tile_pool(name="acts", bufs=2) as apool,
            tc.tile_pool(name="ptile", bufs=6) as ppool,
            tc.tile_pool(name="asmall", bufs=3) as aspool,
            tc.tile_pool(name="obuf", bufs=2) as opool,
            tc.tile_pool(name="psum", bufs=1, space="PSUM") as psum,
        ):
            wq_sb = wpool.tile([P, N_HC, QC], bf16)
            wkv_sb = wpool.tile([P, N_HC, 2 * HD], bf16)
            for hg in range(4):
                hs = slice(hg * 4 * P, (hg + 1) * 4 * P)
                ts = slice(hg * 4, (hg + 1) * 4)
                nc.sync.dma_start(
                    wq_sb[:, ts, :],
                    wq_d[hs, :].rearrange("(hc p) c -> p hc c", p=P))
                nc.sync.dma_start(
                    wkv_sb[:, ts, :],
                    wkv_d[hs, :].rearrange("(hc p) c -> p hc c", p=P))
            wo_sb = wpool.tile([P, 2, H], bf16)  # loaded after first x chunk
            # eye(64) at partitions 64:128 (base partition must match v^T rows)
            ident = wpool.tile([P, HD], bf16)
            nc.gpsimd.memset(ident[:], 0.0)
            make_identity(nc, ident[HD:P, :], nomemset=True)
            # broadcast mask: even-head rcp (row 64) -> out rows 0:64,
            # odd-head rcp (row 0) -> out rows 64:128
            bmask = wpool.tile([P, P], f32r)
            nc.gpsimd.memset(bmask[:].bitcast(f32), 0.0)
            nc.gpsimd.memset(bmask[HD:HD + 1, 0:HD].bitcast(f32), 1.0)
            nc.gpsimd.memset(bmask[0:1, HD:P].bitcast(f32), 1.0)
            # rcp rows 1:64 and 65:128 must stay zero (bmask kills them, but
            # NaN*0 would poison) - zero once, only rows 0 and 64 get written.
            rcp_buf = wpool.tile([P, 2, SQT], f32r)
            nc.vector.memset(rcp_buf[:].bitcast(f32), 0.0)

            prev = None
            for b in range(B):
                # ---------- phase A: projections ----------
                qT = apool.tile([P, 2, S], bf16, tag="qT")
                kvT = apool.tile([P, S], bf16, tag="kvT")  # k rows 0:64, v 64:128
                kdup = apool.tile([P, S], bf16, tag="kdup")  # k at rows 64:128
                vpe = apool.tile([P, N_SKC, HD + 1], bf16, tag="vpe")
                vpo = apool.tile([P, N_SKC, P], bf16, tag="vpo")

                for sh in range(2):
                    xb = xpool.tile([P, N_HC, SH], bf16, tag="xb")
                    for hg in range(4):
                        nc.sync.dma_start(
                            xb[:, hg * 4:(hg + 1) * 4, :],
                            xT_d[b, hg * 4 * P:(hg + 1) * 4 * P,
                                 sh * SH:(sh + 1) * SH].rearrange(
                                "(hc p) s -> p hc s", p=P))
                        if b == 0 and sh == 0 and hg == 0:
                            nc.sync.dma_start(
                                wo_sb[:], wo_d.rearrange("(c p) n -> p c n", p=P))
                    for st in range(2):
                        s0 = sh * SH + st * SQT
                        qp0 = psum.tile([P, SQT], f32, tag="outp", bufs=2, name="qp0")
                        qp1 = psum.tile([P, SQT], f32, tag="outp", bufs=2, name="qp1")
                        kvp = psum.tile([P, SQT], f32, tag="opb", bufs=2, name="kvp")
                        qp = (qp0, qp1)
                        for hc in range(N_HC):
                            rhs = xb[:, hc, st * SQT:(st + 1) * SQT]
                            for cc in range(2):
                                nc.tensor.matmul(
                                    qp[cc], wq_sb[:, hc, cc * P:(cc + 1) * P],
                                    rhs, start=(hc == 0), stop=(hc == N_HC - 1))
                            nc.tensor.matmul(
                                kvp, wkv_sb[:, hc, :], rhs,
                                start=(hc == 0), stop=(hc == N_HC - 1))
                        for cc in range(2):
                            nc.vector.tensor_copy(qT[:, cc, s0:s0 + SQT], qp[cc])
                        nc.vector.tensor_copy(kvT[:, s0:s0 + SQT], kvp)

                # duplicate k to partitions 64:128 for odd-head score matmuls
                nc.sync.dma_start(kdup[HD:P, :], kvT[0:HD, :])

                # V' tiles: transpose v^T via PE.
                # even: [V | ones] (PV rows 0:64, denom 64)
                # odd:  [ones | 0 | V] (denom row 0, PV rows 64:128)
                nc.gpsimd.memset(vpo[:], 0.0)
                nc.vector.memset(vpe[:, :, HD:HD + 1], 1.0)
                nc.vector.memset(vpo[:, :, 0:1], 1.0)
                for t in range(N_SKC):
                    tp = psum.tile([P, HD], bf16, tag="opb", bufs=2)
                    nc.tensor.matmul(tp[:], kvT[HD:P, t * P:(t + 1) * P],
                                     ident[HD:P, :], is_transpose=True)
                    nc.vector.tensor_copy(vpe[:, t, 0:HD], tp[:])
                    nc.vector.tensor_copy(vpo[:, t, HD:P], tp[:])

                # ---------- phase B: attention + out-proj ----------
                # out-proj for block sqt-1 is EMITTED after block sqt's
                # attention: its lower scheduler priority then lets it fill
                # the PE bubbles of the ACT(exp)-limited attention stretch.
                def emit_outproj(aTp, sq0p, bp, sqcs, demote=True):
                  with tc.high_priority(offset=-500000 if demote else 0):
                    for sqc in sqcs:
                        row0 = sq0p + sqc * P
                        ob = opool.tile([P, H], fp16, tag="ob", name="ob")
                        for oc in range(4):
                            op_ = psum.tile([P, SQT], f32, tag="opb", bufs=2,
                                            name="op_")
                            for hdc in range(2):
                                nc.tensor.matmul(
                                    op_, aTp[:, hdc, sqc * P:(sqc + 1) * P],
                                    wo_sb[:, hdc, oc * SQT:(oc + 1) * SQT],
                                    start=(hdc == 0), stop=(hdc == 1))
                            nc.vector.tensor_copy(ob[:, oc * SQT:(oc + 1) * SQT],
                                                  op_)
                        nc.sync.dma_start(out_d[bp, row0:row0 + P, :], ob[:])

                for sqt in range(N_SQT):
                    sq0 = sqt * SQT
                    aTt = aspool.tile([P, 2, SQT], bf16, tag="aT")
                    for pair in range(2):
                        outp_e = psum.tile([P, SQT], f32, tag="outp", bufs=2)
                        outp_o = psum.tile([P, SQT], f32, tag="outp", bufs=2)
                        for parity, outp in ((0, outp_e), (1, outp_o)):
                            lo = parity * HD
                            hi = lo + HD
                            kt = kvT if parity == 0 else kdup
                            qh = qT[lo:hi, pair, sq0:sq0 + SQT]
                            for g2 in range(N_SKC // 2):
                                sgrp = psum.tile([P, 2, SQT], f32, tag="sring", bufs=2)
                                for j in range(2):
                                    sk = g2 * 2 + j
                                    nc.tensor.matmul(
                                        sgrp[:, j, :],
                                        kt[lo:hi, sk * P:(sk + 1) * P], qh,
                                        start=True, stop=True)
                                pt = ppool.tile([P, 2, SQT], bf16, tag="pt")
                                nc.scalar.activation(pt[:], sgrp[:], Exp,
                                                     scale=0.125)
                                for j in range(2):
                                    sk = g2 * 2 + j
                                    vp = (vpe if parity == 0 else vpo)[:, sk, :]
                                    nout = HD + 1 if parity == 0 else P
                                    nc.tensor.matmul(
                                        outp[0:nout, :], vp, pt[:, j, :],
                                        start=(sk == 0), stop=(sk == N_SKC - 1))
                            # normalize this parity now; the rcp/pbr/rb/aT
                            # chain hides under the other parity's attention
                            # and frees this outp psum slot early.
                            # denominators: even at row 64, odd at row 0
                            lo_, dr = (0, HD) if parity == 0 else (HD, 0)
                            with nc.allow_low_precision(reason="f32r recip"):
                                nc.vector.reciprocal(rcp_buf[dr:dr + 1, pair, :],
                                                     outp[dr:dr + 1, :])
                            pbr = psum.tile([P, SQT], f32, tag="opb", bufs=2,
                                            name="pbr")
                            nc.tensor.matmul(
                                pbr[:], bmask[dr:dr + 1, :],
                                rcp_buf[dr:dr + 1, pair, :],
                                start=True, stop=True)
                            rb = aspool.tile([P, SQT], f32, tag="rb", name="rb")
                            nc.vector.tensor_copy(rb[lo_:lo_ + HD, :],
                                                  pbr[lo_:lo_ + HD, :])
                            nc.vector.tensor_tensor(
                                aTt[lo_:lo_ + HD, pair, :], outp[lo_:lo_ + HD, :],
                                rb[lo_:lo_ + HD, :], op=mult)
                        if prev is not None:
                            emit_outproj(*prev, (0, 1) if pair == 0 else (2, 3))
                    prev = (aTt, sq0, b)
            emit_outproj(*prev, (0, 1, 2, 3), demote=False)
    nc.compile()
    return nc


def core_assignment(c):
    """(q_heads, kv_heads, batches, wo_row_slice) owned by core c."""
    return (list(range(G * c, G * c + G)), [c], list(range(B)),
            slice(c * QC, (c + 1) * QC))


def make_in_maps(inputs):
    from ml_dtypes import bfloat16

    x = np.asarray(inputs["x"], dtype=np.float32)
    Wq = np.asarray(inputs["Wq"], dtype=np.float32)
    Wk = np.asarray(inputs["Wk"], dtype=np.float32)
    Wv = np.asarray(inputs["Wv"], dtype=np.float32)
    Wo = np.asarray(inputs["Wo"], dtype=np.float32)

    xT = np.ascontiguousarray(x.transpose(0, 2, 1)).astype(bfloat16)
    in_maps = []
    for c in range(NCORES):
        wq_c = np.ascontiguousarray(Wq[:, c * QC:(c + 1) * QC]).astype(bfloat16)
        wkv_c = np.concatenate(
            [Wk[:, c * HD:(c + 1) * HD], Wv[:, c * HD:(c + 1) * HD]],
            axis=1).astype(bfloat16)
        wo_c = np.ascontiguousarray(Wo[c * QC:(c + 1) * QC, :]).astype(bfloat16)
        in_maps.append({"xT": xT, "wq": wq_c, "wkv": wkv_c, "wo": wo_c})
    return in_maps


def kernel(**inputs):
    from concourse.bass_utils import run_bass_kernel_spmd

    bo = np.asarray(inputs["bo"], dtype=np.float32)
    in_maps = make_in_maps(inputs)

    if "nc" not in _cached:
        _cached["nc"] = _build_nc()
    trace = bool(int(os.environ.get("GQA_TRACE", "0")))
    res = run_bass_kernel_spmd(_cached["nc"], in_maps, list(range(NCORES)),
                               trace=trace)
    _cached["last_result"] = res
    out = res.results[0]["out"].astype(np.float32)
    for c in range(1, NCORES):
        out += res.results[c]["out"].astype(np.float32)
    out += bo
    return out
